# revision 25
# baseline (speedup 1.0000x reference)
"""BinaryConv2d (3x3, SAME, NHWC) on 8 trn2 NeuronCores.

Sharding: data-parallel over batch — 2 images per core; the tiny binarized
weight tensor is replicated. Per core, the two images are packed on the two
64-partition halves of SBUF so each tap-matmul pair (K=64 contraction = C_in)
runs concurrently on disjoint row-groups of the 128x128 PE array.

Layout trick: the conv is evaluated on a flat q-grid over a zero-padded
226-wide plane, so each of the 9 taps is a pure free-dim offset
(dh*226 + dw) into the same SBUF x tile; row-crossing outputs land in 2
garbage columns per row that the host discards.

All-fp8 DoubleRow pipeline: every matmul is fp8e4 with perf_mode=DoubleRow
(2 MACs/cell/cycle), so a 512-output tap-pair matmul streams in 512 cycles
— 7 matmuls per slot instead of 9:
  - 2 "speed" matmuls cover taps (0,0)+(0,1) and (1,0)+(1,1): the pair dim
    reads plane rows 0 and 2 of a [128, 3, C] tile (row2 = x8 shifted +1
    col), giving each tap plain-e4m3 precision.
  - 5 "accurate" matmuls cover the remaining taps as (x8, r8) pairs where
    r8 = e4m3(16*(x - x8)) lives in plane row 1 and the second weight is
    W*2^-4 (exact in e4m3) — effectively ~11-bit input precision.
DoubleRow pair strides must be 16B-aligned (stride-1 pairs fault), hence
the shifted-copy row. Measured end-to-end rel err vs the fp32 reference is
~1.78e-2, dominated by e4m3 quantization of the 4 speed taps.
"""

import sys

for _p in ("/opt/trn_rl_repo",):
    if _p not in sys.path:
        sys.path.insert(0, _p)

import ml_dtypes
import numpy as np

BF16 = ml_dtypes.bfloat16
E4M3 = ml_dtypes.float8_e4m3

N_CORES = 8
IMG_PER_CORE = 2
H = W_IMG = 224
C_IN, C_OUT = 64, 128
PR, PC = 227, 226  # padded plane: 226 rows of data + 1 extra zero row
PLANE = PR * PC  # 51302
QOUT = H * PC  # 50624 q-positions per image (2 garbage cols per row)
SLOT = 512  # q-positions per matmul (one PSUM bank of fp32)
N_SLOTS = (QOUT + SLOT - 1) // SLOT  # 99 (last slot = 448)
SLOTS_PER_CHUNK = 16
HALO = 2 * PC + 2  # 454: max tap offset
CHUNK_Q = SLOTS_PER_CHUNK * SLOT
XTILE_COLS = CHUNK_Q + HALO
XT8_COLS = ((XTILE_COLS + 1 + 15) // 16) * 16  # 16B-aligned DoubleRow pair stride
STAGE_SLOTS = 8
STAGE_Q = STAGE_SLOTS * SLOT

# Speed pairs: taps (dh,0)+(dh,1) for dh=0,1 -> base offsets; pair = rows (0,2).
SPEED_PAIRS = [0 * PC, 1 * PC]
# Accurate taps (x8 + r8/16): the remaining five -> base offsets; pair = rows (0,1).
ACC_TAPS = [0 * PC + 2, 1 * PC + 2, 2 * PC + 0, 2 * PC + 1, 2 * PC + 2]


def _chunk_plan():
    """(start_slot -> n_slots): geometric ramp so early chunks land
    just-in-time, then steady 16-slot chunks."""
    plan = {}
    s, size = 0, 1
    while s < N_SLOTS:
        n = min(size, N_SLOTS - s, SLOTS_PER_CHUNK)
        plan[s] = n
        s += n
        size *= 2
    return plan


def _stage_plan():
    """(start_slot -> n_slots): 8-slot stages, then 2-slot stages for the
    last ~11 so the out-DMA backlog after the final matmul is tiny."""
    plan = {}
    s = 0
    while s < N_SLOTS:
        rem = N_SLOTS - s
        if rem > 11:
            n = STAGE_SLOTS
        elif rem > 2:
            n = 2
        else:
            n = rem
        plan[s] = n
        s += n
    return plan

_COMPILED = None
_LAST_RES = None


def _build():
    import concourse.mybir as mybir
    import concourse.tile as tile
    from concourse import bacc

    nc = bacc.Bacc(
        "TRN2", target_bir_lowering=False, debug=False, num_devices=N_CORES
    )
    # Three interleaved fp8 planes per partition: x8, r8, x8 shifted +1 col.
    x8_d = nc.dram_tensor(
        "x8", [128, 3 * PLANE], mybir.dt.float8e4, kind="ExternalInput"
    )
    NPAIR = len(SPEED_PAIRS) + len(ACC_TAPS)  # 7
    w8_d = nc.dram_tensor(
        "w8", [128, NPAIR * 2 * 128], mybir.dt.float8e4, kind="ExternalInput"
    )
    b_d = nc.dram_tensor("b", [128, 1], mybir.dt.float32, kind="ExternalInput")
    o_d = nc.dram_tensor(
        "out", [128, IMG_PER_CORE * QOUT], mybir.dt.bfloat16, kind="ExternalOutput"
    )

    ident = mybir.ActivationFunctionType.Identity
    DR = mybir.MatmulPerfMode.DoubleRow
    assert XT8_COLS % 16 == 0 and (2 * XT8_COLS) % 16 == 0

    with tile.TileContext(nc) as tc:
        with (
            tc.tile_pool(name="const", bufs=1) as cpool,
            tc.tile_pool(name="x8in", bufs=5) as x8pool,
            tc.tile_pool(name="stage", bufs=3) as spool,
            tc.tile_pool(name="psum", bufs=3, space="PSUM") as ppool,
        ):
            # Critical-path-first ordering on the HWDGE ring: weights, first
            # small x chunk, bias, then geometrically ramped x chunks.
            w8_sb = cpool.tile([128, NPAIR, 2, 128], mybir.dt.float8e4, tag="w8")
            nc.sync.dma_start(
                w8_sb[:, :, :, :],
                w8_d[:, :].rearrange(
                    "p (pair two m) -> p pair two m", pair=NPAIR, two=2
                ),
            )
            b_sb = cpool.tile([128, 1], mybir.dt.float32, tag="b")

            # One HAM activity window (~3.4us) of dummy cold matmuls on a
            # zeroed tile, sized to finish as the first x chunk lands: the
            # PE clock-gate releases before the real stream starts, so it
            # runs at 2.4GHz from matmul 0 (results are never read).
            warm_src = cpool.tile([128, SLOT], mybir.dt.bfloat16, tag="warm")
            nc.vector.memset(warm_src[:], 0.0)
            warm_ps = ppool.tile([128, SLOT], mybir.dt.float32, tag="pswarm", bufs=1)
            N_WARM = 8
            for i in range(N_WARM):
                nc.tensor.matmul(
                    warm_ps[:, :],
                    lhsT=warm_src[:, 0:128],
                    rhs=warm_src[:, :],
                    start=(i == 0),
                    stop=(i == N_WARM - 1),
                )

            chunk_plan = _chunk_plan()
            stage_plan = _stage_plan()
            xt8 = None
            st_a = st_b = None
            stage_end = -1
            for s in range(N_SLOTS):
                q0 = s * SLOT
                n = min(SLOT, QOUT - q0)

                if s in chunk_plan:
                    cq0 = q0
                    ext = min(QOUT, cq0 + chunk_plan[s] * SLOT) - cq0 + HALO
                    xt8 = x8pool.tile([128, 3, XT8_COLS], mybir.dt.float8e4, tag="x8")
                    src = x8_d[:, :].rearrange("p (three c) -> p three c", three=3)
                    nc.sync.dma_start(xt8[:, :, :ext], src[:, :, cq0 : cq0 + ext])
                    if s == 0:
                        nc.sync.dma_start(b_sb[:], b_d[:])

                if s in stage_plan:
                    g0 = q0
                    gext = min(QOUT, g0 + stage_plan[s] * SLOT) - g0
                    stage_end = s + stage_plan[s] - 1
                    st_a = spool.tile([128, STAGE_Q], mybir.dt.bfloat16, tag="sa")
                    st_b = spool.tile([128, STAGE_Q], mybir.dt.bfloat16, tag="sb")

                psa = ppool.tile([128, SLOT], mybir.dt.float32, tag="psa")
                psb = ppool.tile([128, SLOT], mybir.dt.float32, tag="psb")

                lo0 = q0 - cq0
                for h, ps in ((0, psa), (64, psb)):
                    # Speed pairs: taps (dh,0)+(dh,1) via plane rows (0,2).
                    for pi, poff in enumerate(SPEED_PAIRS):
                        lo = lo0 + poff
                        rhs = xt8[h : h + 64, 0, lo : lo + n].unsqueeze(1)
                        rhs.ap[1] = [2 * XT8_COLS, 2]
                        nc.tensor.matmul(
                            ps[:, :n],
                            lhsT=w8_sb[h : h + 64, pi, :, :],
                            rhs=rhs,
                            start=(pi == 0),
                            stop=False,
                            perf_mode=DR,
                        )
                    # Accurate taps: (x8, r8) residual pairs via rows (0,1).
                    for ti, toff in enumerate(ACC_TAPS):
                        lo = lo0 + toff
                        nc.tensor.matmul(
                            ps[:, :n],
                            lhsT=w8_sb[h : h + 64, len(SPEED_PAIRS) + ti, :, :],
                            rhs=xt8[h : h + 64, 0:2, lo : lo + n],
                            start=False,
                            stop=(ti == len(ACC_TAPS) - 1),
                            perf_mode=DR,
                        )

                so = q0 - g0
                nc.vector.tensor_scalar_add(st_a[:, so : so + n], psa[:, :n], b_sb[:])
                nc.scalar.activation(st_b[:, so : so + n], psb[:, :n], ident, bias=b_sb[:])

                if s == stage_end:
                    nc.sync.dma_start(o_d[:, g0 : g0 + gext], st_a[:, :gext])
                    nc.sync.dma_start(
                        o_d[:, QOUT + g0 : QOUT + g0 + gext], st_b[:, :gext]
                    )

    nc.compile()
    return nc


def _get_nc():
    global _COMPILED
    if _COMPILED is None:
        _COMPILED = _build()
    return _COMPILED


def kernel(x: np.ndarray, W: np.ndarray, b: np.ndarray) -> np.ndarray:
    from concourse.bass_utils import run_bass_kernel_spmd

    nc = _get_nc()

    xf = np.asarray(x, dtype=np.float32)
    xr = xf.reshape(N_CORES, IMG_PER_CORE, H, W_IMG, C_IN).transpose(0, 1, 4, 2, 3)
    x8r = xr.astype(E4M3)
    r8r = (16.0 * (xr - x8r.astype(np.float32))).astype(E4M3)
    XB = np.zeros((N_CORES, IMG_PER_CORE * C_IN, 3, PR * PC), E4M3)
    XBv = XB.reshape(N_CORES, IMG_PER_CORE, C_IN, 3, PR, PC)
    XBv[:, :, :, 0, 1 : H + 1, 1 : W_IMG + 1] = x8r
    XBv[:, :, :, 1, 1 : H + 1, 1 : W_IMG + 1] = r8r
    # plane row 2 = x8 shifted one column left (pair element for tap (dh,1))
    XB[:, :, 2, :-1] = XB[:, :, 0, 1:]
    X8f = XB.reshape(N_CORES, 128, 3 * PLANE)

    Wb = np.sign(np.asarray(W, dtype=np.float32)).reshape(9, C_IN, C_OUT)
    # 7 DoubleRow pairs: 2 speed pairs (taps 0+1, 3+4), then 5 accurate taps
    # (2,5,6,7,8) paired with their own weights scaled by 2^-4 (residual).
    NPAIR = 7
    wp = np.empty((NPAIR, 2, C_IN, C_OUT), np.float32)
    wp[0, 0], wp[0, 1] = Wb[0], Wb[1]
    wp[1, 0], wp[1, 1] = Wb[3], Wb[4]
    for i, t in enumerate([2, 5, 6, 7, 8]):
        wp[2 + i, 0] = Wb[t]
        wp[2 + i, 1] = Wb[t] * (1.0 / 16.0)
    w8h = np.empty((2, C_IN, NPAIR, 2, C_OUT), E4M3)
    w8h[:] = wp.transpose(2, 0, 1, 3)[None].astype(E4M3)
    w8h = np.ascontiguousarray(w8h.reshape(128, NPAIR * 2 * C_OUT))

    bh = np.ascontiguousarray(np.asarray(b, dtype=np.float32).reshape(128, 1))

    in_maps = [{"x8": X8f[c], "w8": w8h, "b": bh} for c in range(N_CORES)]
    res = run_bass_kernel_spmd(nc, in_maps, list(range(N_CORES)))
    global _LAST_RES
    _LAST_RES = res

    O = np.stack([res.results[c]["out"] for c in range(N_CORES)])
    O = O.reshape(N_CORES, C_OUT, IMG_PER_CORE, H, PC)[:, :, :, :, :W_IMG]
    y = O.transpose(0, 2, 3, 4, 1).reshape(16, H, W_IMG, C_OUT)
    return np.ascontiguousarray(y).astype(np.float32)


# revision 29
# speedup vs baseline: 1.1623x; 1.1623x over previous
"""BinaryConv2d (3x3, SAME, NHWC) on 8 trn2 NeuronCores.

Sharding: data-parallel over batch — 2 images per core; the tiny binarized
weight tensor is replicated. Per core, the two images are packed on the two
64-partition halves of SBUF so each tap-matmul pair (K=64 contraction = C_in)
runs concurrently on disjoint row-groups of the 128x128 PE array.

Layout trick: the conv is evaluated on a flat q-grid over a zero-padded
226-wide plane, so each of the 9 taps is a pure free-dim offset
(dh*226 + dw) into the same SBUF x tile; row-crossing outputs land in 2
garbage columns per row that the host discards.

All-fp8 DoubleRow pipeline: every matmul is fp8e4 with perf_mode=DoubleRow
(2 MACs/cell/cycle), so a 512-output tap-pair matmul streams in 512 cycles
— 7 matmuls per slot instead of 9:
  - 2 "speed" matmuls cover taps (0,0)+(0,1) and (1,0)+(1,1): the pair dim
    reads plane rows 0 and 2 of a [128, 3, C] tile (row2 = x8 shifted +1
    col), giving each tap plain-e4m3 precision.
  - 5 "accurate" matmuls cover the remaining taps as (x8, r8) pairs where
    r8 = e4m3(16*(x - x8)) lives in plane row 1 and the second weight is
    W*2^-4 (exact in e4m3) — effectively ~11-bit input precision.
DoubleRow pair strides must be 16B-aligned (stride-1 pairs fault), hence
the shifted-copy row. Measured end-to-end rel err vs the fp32 reference is
~1.78e-2, dominated by e4m3 quantization of the 4 speed taps.
"""

import sys

for _p in ("/opt/trn_rl_repo",):
    if _p not in sys.path:
        sys.path.insert(0, _p)

import ml_dtypes
import numpy as np

BF16 = ml_dtypes.bfloat16
E4M3 = ml_dtypes.float8_e4m3

N_CORES = 8
IMG_PER_CORE = 2
H = W_IMG = 224
C_IN, C_OUT = 64, 128
PR, PC = 227, 226  # padded plane: 226 rows of data + 1 extra zero row
PLANE = PR * PC  # 51302
QOUT = H * PC  # 50624 q-positions per image (2 garbage cols per row)
SLOT = 512  # q-positions per matmul (one PSUM bank of fp32)
N_SLOTS = (QOUT + SLOT - 1) // SLOT  # 99 (last slot = 448)
SLOTS_PER_CHUNK = 16
HALO = 2 * PC + 2  # 454: max tap offset
CHUNK_Q = SLOTS_PER_CHUNK * SLOT
XTILE_COLS = CHUNK_Q + HALO
XT8_COLS = ((XTILE_COLS + 1 + 15) // 16) * 16  # 16B-aligned DoubleRow pair stride
STAGE_SLOTS = 8
STAGE_Q = STAGE_SLOTS * SLOT

# Speed pairs: taps (dh,0)+(dh,1) for dh=0,1 -> base offsets; pair = rows (0,2).
SPEED_PAIRS = [0 * PC, 1 * PC]
# Accurate taps (x8 + r8/16): the remaining five -> base offsets; pair = rows (0,1).
ACC_TAPS = [0 * PC + 2, 1 * PC + 2, 2 * PC + 0, 2 * PC + 1, 2 * PC + 2]


def _chunk_plan():
    """(start_slot -> n_slots): geometric ramp so early chunks land
    just-in-time, then steady 16-slot chunks."""
    plan = {}
    s, size = 0, 1
    while s < N_SLOTS:
        n = min(size, N_SLOTS - s, SLOTS_PER_CHUNK)
        plan[s] = n
        s += n
        size *= 2
    return plan


def _stage_plan():
    """(start_slot -> n_slots): 8-slot stages, then 2-slot stages for the
    last ~11 so the out-DMA backlog after the final matmul is tiny."""
    plan = {}
    s = 0
    while s < N_SLOTS:
        rem = N_SLOTS - s
        if rem > 11:
            n = STAGE_SLOTS
        elif rem > 2:
            n = 2
        else:
            n = rem
        plan[s] = n
        s += n
    return plan

_COMPILED = None
_LAST_RES = None


def _build():
    import concourse.mybir as mybir
    import concourse.tile as tile
    from concourse import bacc

    nc = bacc.Bacc(
        "TRN2", target_bir_lowering=False, debug=False, num_devices=N_CORES
    )
    x8_d = nc.dram_tensor("x8", [128, PLANE], mybir.dt.float8e4, kind="ExternalInput")
    r8_d = nc.dram_tensor("r8", [128, PLANE], mybir.dt.float8e4, kind="ExternalInput")
    NPAIR = len(SPEED_PAIRS) + len(ACC_TAPS)  # 7
    w8_d = nc.dram_tensor(
        "w8", [128, NPAIR * 2 * 128], mybir.dt.float8e4, kind="ExternalInput"
    )
    b_d = nc.dram_tensor("b", [128, 1], mybir.dt.float32, kind="ExternalInput")
    o_d = nc.dram_tensor(
        "out", [128, IMG_PER_CORE * QOUT], mybir.dt.bfloat16, kind="ExternalOutput"
    )

    ident = mybir.ActivationFunctionType.Identity
    DR = mybir.MatmulPerfMode.DoubleRow
    assert XT8_COLS % 16 == 0 and (2 * XT8_COLS) % 16 == 0

    with tile.TileContext(nc) as tc:
        with (
            tc.tile_pool(name="const", bufs=1) as cpool,
            tc.tile_pool(name="x8in", bufs=5) as x8pool,
            tc.tile_pool(name="stage", bufs=3) as spool,
            tc.tile_pool(name="psum", bufs=3, space="PSUM") as ppool,
        ):
            # Critical-path-first ordering on the HWDGE ring: weights, first
            # small x chunk, bias, then geometrically ramped x chunks.
            w8_sb = cpool.tile([128, NPAIR, 2, 128], mybir.dt.float8e4, tag="w8")
            nc.sync.dma_start(
                w8_sb[:, :, :, :],
                w8_d[:, :].rearrange(
                    "p (pair two m) -> p pair two m", pair=NPAIR, two=2
                ),
            )
            b_sb = cpool.tile([128, 1], mybir.dt.float32, tag="b")

            # One HAM activity window (~3.4us) of dummy cold matmuls on a
            # zeroed tile, sized to finish as the first x chunk lands: the
            # PE clock-gate releases before the real stream starts, so it
            # runs at 2.4GHz from matmul 0 (results are never read).
            warm_src = cpool.tile([128, SLOT], mybir.dt.bfloat16, tag="warm")
            nc.vector.memset(warm_src[:], 0.0)
            warm_ps = ppool.tile([128, SLOT], mybir.dt.float32, tag="pswarm", bufs=1)
            N_WARM = 8
            for i in range(N_WARM):
                nc.tensor.matmul(
                    warm_ps[:, :],
                    lhsT=warm_src[:, 0:128],
                    rhs=warm_src[:, :],
                    start=(i == 0),
                    stop=(i == N_WARM - 1),
                )

            chunk_plan = _chunk_plan()
            stage_plan = _stage_plan()
            xt8 = None
            st_a = st_b = None
            stage_end = -1
            for s in range(N_SLOTS):
                q0 = s * SLOT
                n = min(SLOT, QOUT - q0)

                if s in chunk_plan:
                    cq0 = q0
                    ext = min(QOUT, cq0 + chunk_plan[s] * SLOT) - cq0 + HALO
                    xt8 = x8pool.tile([128, 3, XT8_COLS], mybir.dt.float8e4, tag="x8")
                    nc.sync.dma_start(xt8[:, 0, :ext], x8_d[:, cq0 : cq0 + ext])
                    nc.sync.dma_start(xt8[:, 1, :ext], r8_d[:, cq0 : cq0 + ext])
                    nc.sync.dma_start(xt8[:, 2, :ext], x8_d[:, cq0 + 1 : cq0 + 1 + ext])
                    if s == 0:
                        nc.sync.dma_start(b_sb[:], b_d[:])

                if s in stage_plan:
                    g0 = q0
                    gext = min(QOUT, g0 + stage_plan[s] * SLOT) - g0
                    stage_end = s + stage_plan[s] - 1
                    st_a = spool.tile([128, STAGE_Q], mybir.dt.bfloat16, tag="sa")
                    st_b = spool.tile([128, STAGE_Q], mybir.dt.bfloat16, tag="sb")

                psa = ppool.tile([128, SLOT], mybir.dt.float32, tag="psa")
                psb = ppool.tile([128, SLOT], mybir.dt.float32, tag="psb")

                lo0 = q0 - cq0
                for h, ps in ((0, psa), (64, psb)):
                    # Speed pairs: taps (dh,0)+(dh,1) via plane rows (0,2).
                    for pi, poff in enumerate(SPEED_PAIRS):
                        lo = lo0 + poff
                        rhs = xt8[h : h + 64, 0, lo : lo + n].unsqueeze(1)
                        rhs.ap[1] = [2 * XT8_COLS, 2]
                        nc.tensor.matmul(
                            ps[:, :n],
                            lhsT=w8_sb[h : h + 64, pi, :, :],
                            rhs=rhs,
                            start=(pi == 0),
                            stop=False,
                            perf_mode=DR,
                        )
                    # Accurate taps: (x8, r8) residual pairs via rows (0,1).
                    for ti, toff in enumerate(ACC_TAPS):
                        lo = lo0 + toff
                        nc.tensor.matmul(
                            ps[:, :n],
                            lhsT=w8_sb[h : h + 64, len(SPEED_PAIRS) + ti, :, :],
                            rhs=xt8[h : h + 64, 0:2, lo : lo + n],
                            start=False,
                            stop=(ti == len(ACC_TAPS) - 1),
                            perf_mode=DR,
                        )

                so = q0 - g0
                nc.vector.tensor_scalar_add(st_a[:, so : so + n], psa[:, :n], b_sb[:])
                nc.scalar.activation(st_b[:, so : so + n], psb[:, :n], ident, bias=b_sb[:])

                if s == stage_end:
                    nc.sync.dma_start(o_d[:, g0 : g0 + gext], st_a[:, :gext])
                    nc.sync.dma_start(
                        o_d[:, QOUT + g0 : QOUT + g0 + gext], st_b[:, :gext]
                    )

    nc.compile()
    return nc


def _get_nc():
    global _COMPILED
    if _COMPILED is None:
        _COMPILED = _build()
    return _COMPILED


def kernel(x: np.ndarray, W: np.ndarray, b: np.ndarray) -> np.ndarray:
    from concourse.bass_utils import run_bass_kernel_spmd

    nc = _get_nc()

    xf = np.asarray(x, dtype=np.float32)
    xr = xf.reshape(N_CORES, IMG_PER_CORE, H, W_IMG, C_IN).transpose(0, 1, 4, 2, 3)
    x8r = xr.astype(E4M3)
    r8r = (16.0 * (xr - x8r.astype(np.float32))).astype(E4M3)
    X8 = np.zeros((N_CORES, IMG_PER_CORE, C_IN, PR, PC), E4M3)
    X8[:, :, :, 1 : H + 1, 1 : W_IMG + 1] = x8r
    X8f = X8.reshape(N_CORES, 128, PLANE)
    R8 = np.zeros((N_CORES, IMG_PER_CORE, C_IN, PR, PC), E4M3)
    R8[:, :, :, 1 : H + 1, 1 : W_IMG + 1] = r8r
    R8f = R8.reshape(N_CORES, 128, PLANE)

    Wb = np.sign(np.asarray(W, dtype=np.float32)).reshape(9, C_IN, C_OUT)
    # 7 DoubleRow pairs: 2 speed pairs (taps 0+1, 3+4), then 5 accurate taps
    # (2,5,6,7,8) paired with their own weights scaled by 2^-4 (residual).
    NPAIR = 7
    wp = np.empty((NPAIR, 2, C_IN, C_OUT), np.float32)
    wp[0, 0], wp[0, 1] = Wb[0], Wb[1]
    wp[1, 0], wp[1, 1] = Wb[3], Wb[4]
    for i, t in enumerate([2, 5, 6, 7, 8]):
        wp[2 + i, 0] = Wb[t]
        wp[2 + i, 1] = Wb[t] * (1.0 / 16.0)
    w8h = np.empty((2, C_IN, NPAIR, 2, C_OUT), E4M3)
    w8h[:] = wp.transpose(2, 0, 1, 3)[None].astype(E4M3)
    w8h = np.ascontiguousarray(w8h.reshape(128, NPAIR * 2 * C_OUT))

    bh = np.ascontiguousarray(np.asarray(b, dtype=np.float32).reshape(128, 1))

    in_maps = [
        {"x8": X8f[c], "r8": R8f[c], "w8": w8h, "b": bh} for c in range(N_CORES)
    ]
    res = run_bass_kernel_spmd(nc, in_maps, list(range(N_CORES)))
    global _LAST_RES
    _LAST_RES = res

    O = np.stack([res.results[c]["out"] for c in range(N_CORES)])
    O = O.reshape(N_CORES, C_OUT, IMG_PER_CORE, H, PC)[:, :, :, :, :W_IMG]
    y = O.transpose(0, 2, 3, 4, 1).reshape(16, H, W_IMG, C_OUT)
    return np.ascontiguousarray(y).astype(np.float32)
